# revision 1
# baseline (speedup 1.0000x reference)
"""TRN2 Bass kernel for nn_CrossModalAttention_75316546503126.

Mathematical collapse exploited here (verified against the jax reference):
K/V rows of the attention are identical across the sequence axis because the
acoustic features are broadcast before the K/V projections.  Hence every
attention row sees a constant score vector, softmax is exactly uniform
(S = 2048 is a power of two, so 1/S is exact in fp32), and

    attn_out[b, s, :] = v_b          with  v_b = (ac_b @ Wa + ba) @ Wv + bv
    out[b, s, :]      = text[b, s, :] @ Wt + (bt + bv + fa_b @ Wv)

i.e. one [S, D] x [D, D] matmul per batch plus a per-batch bias row.
Q/K projections cancel entirely.

Sharding: data-parallel over batch B=8 across the 8 NeuronCores (core b
owns batch b).

Implementation notes (driven by NTFF profiles):
  - bf16 everywhere big: single-pass PE rate (fp32r issues HIGH+LOW passes
    at half rate), half the DMA bytes, half the eviction bytes.  Output is
    stored bf16 and upcast on the host; max-rel error ~4e-3 vs the 2e-2
    gate.
  - The host pre-packs X^T, Wt, Wv into the exact [128, free] SBUF tile
    layouts (partition-major), so every big DMA moves 3-9 KB of contiguous
    bytes per partition (~380 GB/s) instead of 1-1.5 KB packets.
  - The DMA queues only come alive ~8.5us into the NEFF (DGE pipe-fill,
    after a ~3.5us engine barrier), and the two hardware queues arbitrate
    at whole-DMA granularity, so the critical chain rides the sync queue
    in exact consumption order: Wt(cols 0:512), X chunk 0, Wt(cols
    512:768), X chunk 1, Wv, X chunks 2..7.  Tiny tensors + output stores
    ride the scalar queue.  Each s-tile issues its six N=512 matmuls
    before its six N=256 ones so tile 0 starts on wt_lo alone.
  - The PE clock (HAM) ramps 0.65 -> 1.2 -> 2.4 GHz only after ~3us of
    gap-free execution and any stall resets it.  WARMUP_MM dummy matmuls
    bridge the dead window -- emitted as ONE long PSUM accumulation group,
    because back-to-back start/stop groups on the same PSUM bank force
    pipeline drains (the gaps crippled the ramp in earlier versions).
  - bias row (bt + bv + fa @ Wv) is built in PSUM via a K=2 ones-matmul
    plus 6 accumulated broadcast(fa^T_k) @ Wv_k products, then fused into
    the PSUM->SBUF eviction add on DVE.  The first N_DEFER tiles are
    evicted bias-less and patched one-per-tile alongside later tiles so
    the DVE keeps pace with the PE.  The last s-tile is evicted and
    stored in two halves on both queues to shorten the tail drain.
"""
import sys

if "/opt/trn_rl_repo" not in sys.path:
    sys.path.insert(0, "/opt/trn_rl_repo")

from contextlib import ExitStack

import numpy as np
import ml_dtypes

import concourse.bacc as bacc
import concourse.bass as bass
import concourse.mybir as mybir
import concourse.tile as tile
from concourse.bass_utils import run_bass_kernel_spmd

F32 = mybir.dt.float32
BF16 = mybir.dt.bfloat16

B, S, D = 8, 2048, 768
KB = D // 128          # 6 contraction blocks
ST = S // 128          # 16 sequence tiles per core
NC_CHUNK = 8           # X^T column chunks
CW = S // NC_CHUNK     # 256 columns per chunk
N_CORES = 8

N_DEFER = 4            # s-tiles evicted before the bias row exists
WARMUP_MM = 16         # dummy [128,512] matmuls that ramp the PE clock

MODE = "bf16"


def build_program(mode=MODE):
    nc = bacc.Bacc()

    # xt is host-packed: xt[p, c*(KB*CW) + k*CW + s] = X[c*CW+s, k*128+p]
    xt = nc.declare_dram_parameter("xt", [128, KB * S], BF16, isOutput=False)
    ac = nc.declare_dram_parameter("ac", [1, 16], F32, isOutput=False)
    # wt is host-packed column-split: [128, 0:3072] holds Wt[:, 0:512]
    # (k-major), [128, 3072:4608] holds Wt[:, 512:768]
    wt = nc.declare_dram_parameter("wt", [128, KB * D], BF16, isOutput=False)
    wa = nc.declare_dram_parameter("wa", [16, D], F32, isOutput=False)
    # wv is host-packed k-major: wv[p, k*768 + d] = Wv[k*128+p, d]
    wv = nc.declare_dram_parameter("wv", [128, KB * D], BF16, isOutput=False)
    bt = nc.declare_dram_parameter("bt", [D], F32, isOutput=False)
    ba = nc.declare_dram_parameter("ba", [D], F32, isOutput=False)
    bv = nc.declare_dram_parameter("bv", [D], F32, isOutput=False)
    out = nc.declare_dram_parameter("out", [S, D], BF16, isOutput=True)

    with tile.TileContext(nc) as tc, ExitStack() as ctx:
        const = ctx.enter_context(tc.tile_pool(name="const", bufs=1))
        wpool = ctx.enter_context(tc.tile_pool(name="wpool", bufs=1))
        xpool = ctx.enter_context(tc.tile_pool(name="xpool", bufs=1))
        dpool = ctx.enter_context(tc.tile_pool(name="dpool", bufs=1))
        opool = ctx.enter_context(tc.tile_pool(name="opool", bufs=3))
        # PSUM (8 banks): 3 x [128,768] out tiles = 6 banks, 1 x [128,768]
        # setup/warm tile = 2 banks
        pso = ctx.enter_context(tc.tile_pool(name="pso", bufs=3, space="PSUM"))
        pset = ctx.enter_context(tc.tile_pool(name="pset", bufs=1, space="PSUM"))

        # ---------------- PE warm-up fodder (no DMA dependencies) --------
        warm_w = const.tile([128, 128], BF16)
        nc.gpsimd.memset(warm_w[:], 1.0)
        warm_x = const.tile([128, 512], BF16)
        nc.gpsimd.memset(warm_x[:], 1.0)
        ones2 = const.tile([2, 128], BF16)
        nc.gpsimd.memset(ones2[:], 1.0)

        # ---------------- DMA schedule ----------------
        # wt_lo leads the SCALAR queue while X chunk 0 leads the SYNC
        # queue: if the two DGE queues can overlap large transfers, the
        # critical lead-in halves; if they arbitrate serially this is
        # equivalent to the old single-queue order.  sync then carries
        # wt_hi, xc1, wv, xc2..xc7; scalar carries the tiny tensors and
        # the output stores behind wt_lo.
        wt_lo = wpool.tile([128, KB * 512], BF16, tag="wtlo", name="wt_lo")
        nc.scalar.dma_start(wt_lo[:], wt[:, 0:KB * 512])

        xc = []
        for c in range(NC_CHUNK):
            t = xpool.tile([128, KB * CW], BF16, tag=f"xc{c}", name=f"xc{c}")
            xc.append(t)

        def load_chunk(c):
            nc.sync.dma_start(xc[c][:], xt[:, c * KB * CW:(c + 1) * KB * CW])

        load_chunk(0)
        wt_hi = wpool.tile([128, KB * 256], BF16, tag="wthi", name="wt_hi")
        nc.sync.dma_start(wt_hi[:], wt[:, KB * 512:KB * 768])
        load_chunk(1)
        wv_sb = wpool.tile([128, KB * D], BF16, tag="wv", name="wv_sb")
        nc.sync.dma_start(wv_sb[:], wv[:])
        for c in range(2, NC_CHUNK):
            load_chunk(c)

        # tiny tensors on the scalar queue
        ac_ext = const.tile([17, 1], F32)
        nc.gpsimd.memset(ac_ext[:], 1.0)
        nc.scalar.dma_start(ac_ext[0:16, :], ac.rearrange("o k -> k o"))
        wa_ext = const.tile([17, D], F32)
        nc.scalar.dma_start(wa_ext[0:16, :], wa[:])
        nc.scalar.dma_start(wa_ext[16:17, :], ba.rearrange("(o n) -> o n", o=1))
        b2_stage = const.tile([2, D], F32)
        nc.scalar.dma_start(b2_stage[0:1, :], bt.rearrange("(o n) -> o n", o=1))
        nc.scalar.dma_start(b2_stage[1:2, :], bv.rearrange("(o n) -> o n", o=1))
        bias2 = const.tile([2, D], BF16)
        nc.vector.tensor_copy(bias2[:], b2_stage[:])

        def x_slice(j, k):
            c, off = j // 2, (j % 2) * 128
            return xc[c][:, k * CW + off:k * CW + off + 128]

        # ---------------- PE warm-up: ramp HAM to 2.4 GHz ----------------
        # ONE long accumulation group: independent start/stop matmuls on
        # the same PSUM bank would each force a pipeline drain.
        warm_ps = pset.tile([128, D], F32, tag="setup", name="warm_ps")
        for i in range(WARMUP_MM):
            nc.tensor.matmul(warm_ps[:, 0:512], warm_w[:], warm_x[:],
                             start=(i == 0), stop=(i == WARMUP_MM - 1),
                             skip_group_check=True)

        # fa^T = ([ac|1] @ [Wa;ba])^T (fp32, tiny) -- runs inside the
        # warm-up window; only needs the tiny scalar-queue tensors.
        fa_ps = pset.tile([128, D], F32, tag="setup", name="fa_ps")
        for m in range(KB):
            nc.tensor.matmul(
                fa_ps[:, m:m + 1],
                wa_ext[:, m * 128:(m + 1) * 128],
                ac_ext[:, :],
                start=True, stop=True,
            )
        faT = const.tile([128, KB], BF16)
        nc.vector.tensor_copy(faT[:], fa_ps[:, 0:KB])

        # ---------------- main s-tile emitters ----------------
        store_eng = [nc.scalar, nc.sync]

        def emit_lo_group(ps, j):
            for k in range(KB):
                nc.tensor.matmul(ps[:, 0:512], x_slice(j, k),
                                 wt_lo[:, k * 512:(k + 1) * 512],
                                 start=(k == 0), stop=(k == KB - 1))

        def emit_hi_group(ps, j):
            for k in range(KB):
                nc.tensor.matmul(ps[:, 512:768], x_slice(j, k),
                                 wt_hi[:, k * 256:(k + 1) * 256],
                                 start=(k == 0), stop=(k == KB - 1))

        def emit_stile_mm(j):
            # interleaved region groups: both PSUM accumulation groups
            # stay open across the tile -> no pipeline drains
            ps = pso.tile([128, D], F32, tag="po")
            for k in range(KB):
                nc.tensor.matmul(ps[:, 0:512], x_slice(j, k),
                                 wt_lo[:, k * 512:(k + 1) * 512],
                                 start=(k == 0), stop=(k == KB - 1))
                nc.tensor.matmul(ps[:, 512:768], x_slice(j, k),
                                 wt_hi[:, k * 256:(k + 1) * 256],
                                 start=(k == 0), stop=(k == KB - 1))
            return ps

        def evict_fused(j, ps):
            # solo tile: store rides the sync queue (the scalar queue
            # carries the late pair store)
            ot = opool.tile([128, D], BF16, tag="o")
            nc.vector.tensor_add(ot[:], ps[:, 0:D], bias_sb[:])
            nc.sync.dma_start(out[j * 128:(j + 1) * 128, :], ot[:])

        deferred = {}

        def defer_evict(j, ps):
            ot = dpool.tile([128, D], F32, tag=f"def{j}", name=f"def{j}")
            nc.vector.tensor_copy(ot[:], ps[:, 0:D])
            deferred[j] = ot

        def emit_stile_deferred(j):
            defer_evict(j, emit_stile_mm(j))

        def flush_deferred(j):
            # flush a PAIR of deferred tiles as one store
            ot = opool.tile([128, 2 * D], BF16, tag="dflush", bufs=2,
                            name=f"dflush{j}")
            nc.vector.tensor_add(ot[:, 0:D], deferred[j][:], bias_sb[:])
            nc.vector.tensor_add(ot[:, D:2 * D], deferred[j + 1][:], bias_sb[:])
            nc.scalar.dma_start(
                out[j * 128:(j + 2) * 128, :].rearrange("(j p) d -> p j d",
                                                        p=128),
                ot[:].rearrange("p (j d) -> p j d", j=2))

        bias_sb = const.tile([128, D], F32)

        # prologue: tiles 0 and 1 run lo-groups first (wt_hi still in
        # flight), interleaved across the two tiles so the PE never stalls
        ps0 = pso.tile([128, D], F32, tag="po", name="ps_t0")
        emit_lo_group(ps0, 0)
        ps1 = pso.tile([128, D], F32, tag="po", name="ps_t1")
        emit_lo_group(ps1, 1)
        emit_hi_group(ps0, 0)
        defer_evict(0, ps0)
        emit_hi_group(ps1, 1)
        defer_evict(1, ps1)
        for j in range(2, N_DEFER):
            emit_stile_deferred(j)

        # bias tile: (bt + bv) + fa @ Wv, fused in PSUM
        bp = pset.tile([128, D], F32, tag="setup", name="bp")
        for lo, hi in ((0, 512), (512, 768)):
            nc.tensor.matmul(bp[:, lo:hi], ones2[:], bias2[:, lo:hi],
                             start=True, stop=True)
            for k in range(KB):
                nc.tensor.matmul(
                    bp[:, lo:hi],
                    faT[:, k:k + 1].broadcast_to([128, 128]),
                    wv_sb[:, k * D + lo:k * D + hi],
                    start=False, stop=(k == KB - 1),
                    skip_group_check=True,
                )
        nc.vector.tensor_copy(bias_sb[:], bp[:, 0:D])

        # remaining s-tiles; stores go out in 2-tile superblocks (fewer DMA
        # triggers + semaphores), deferred pairs flush interleaved so the
        # DVE keeps pace with the PE.  The final tile stores in two halves
        # on both queues to shorten the tail drain.
        pair = {}

        def evict_pair(j, ps):
            lo = pair.pop(j - 1, None)
            if lo is None:
                pair[j] = ps
                return
            ot = opool.tile([128, 2 * D], BF16, tag="o")
            nc.vector.tensor_add(ot[:, 0:D], lo[:, 0:D], bias_sb[:])
            nc.vector.tensor_add(ot[:, D:2 * D], ps[:, 0:D], bias_sb[:])
            store_eng[(j // 2) % 2].dma_start(
                out[(j - 1) * 128:(j + 1) * 128, :].rearrange(
                    "(j p) d -> p j d", p=128),
                ot[:].rearrange("p (j d) -> p j d", j=2))

        for j in range(N_DEFER, ST - 1):
            ps = emit_stile_mm(j)
            if j == ST - 2:
                evict_fused(j, ps)
            else:
                evict_pair(j, ps)
            if j == N_DEFER + 1:
                flush_deferred(0)
            elif j == N_DEFER + 3:
                flush_deferred(2)

        # last tile: lo group completes first and its half evicts + stores
        # while the hi group still runs in a SEPARATE psum tile (no RW
        # dependency against the eviction) -> minimal tail drain
        j = ST - 1
        ps_lo = pso.tile([128, D], F32, tag="po", name="ps_last_lo")
        emit_lo_group(ps_lo, j)
        ps_hi = pso.tile([128, D], F32, tag="po", name="ps_last_hi")
        ot = opool.tile([128, D], BF16, tag="o")
        nc.vector.tensor_add(ot[:, 0:512], ps_lo[:, 0:512], bias_sb[:, 0:512])
        nc.sync.dma_start(out[j * 128:(j + 1) * 128, 0:512], ot[:, 0:512])
        emit_hi_group(ps_hi, j)
        nc.vector.tensor_add(ot[:, 512:768], ps_hi[:, 512:768],
                             bias_sb[:, 512:768])
        nc.scalar.dma_start(out[j * 128:(j + 1) * 128, 512:768],
                            ot[:, 512:768])

    nc.compile()
    return nc


_PROGRAM_CACHE = {}


def _get_program(mode=None):
    if mode is None:
        mode = MODE
    if mode not in _PROGRAM_CACHE:
        _PROGRAM_CACHE[mode] = build_program(mode)
    return _PROGRAM_CACHE[mode]


def make_in_maps(text_features, acoustic_features, Wt, bt, Wa, ba, Wv, bv):
    """Host-side sharding + layout prep: pack per-batch X^T and the weights
    into the exact partition-major SBUF tile layouts, in bf16."""
    bf16 = ml_dtypes.bfloat16
    text_features = np.asarray(text_features, dtype=np.float32)
    # xt[b, p, c*(KB*CW) + k*CW + s] = X[b, c*CW+s, k*128+p]
    xt_all = (text_features
              .reshape(B, NC_CHUNK, CW, KB, 128)
              .transpose(0, 4, 1, 3, 2)
              .astype(bf16)
              .reshape(B, 128, KB * S))

    Wt = np.asarray(Wt, dtype=np.float32).reshape(KB, 128, D)
    wt_lo = Wt[:, :, 0:512].transpose(1, 0, 2).reshape(128, KB * 512)
    wt_hi = Wt[:, :, 512:768].transpose(1, 0, 2).reshape(128, KB * 256)
    wt_packed = np.concatenate([wt_lo, wt_hi], axis=1).astype(bf16)

    Wv = np.asarray(Wv, dtype=np.float32)
    wv_packed = (Wv.reshape(KB, 128, D).transpose(1, 0, 2)
                 .astype(bf16).reshape(128, KB * D))

    shared = {
        "wt": np.ascontiguousarray(wt_packed),
        "wv": np.ascontiguousarray(wv_packed),
        "wa": np.ascontiguousarray(np.asarray(Wa, dtype=np.float32)),
        "bt": np.ascontiguousarray(np.asarray(bt, dtype=np.float32)),
        "ba": np.ascontiguousarray(np.asarray(ba, dtype=np.float32)),
        "bv": np.ascontiguousarray(np.asarray(bv, dtype=np.float32)),
    }
    acoustic_features = np.ascontiguousarray(
        np.asarray(acoustic_features, dtype=np.float32))
    in_maps = []
    for b in range(N_CORES):
        m = dict(shared)
        m["xt"] = np.ascontiguousarray(xt_all[b])
        m["ac"] = acoustic_features[b:b + 1]
        in_maps.append(m)
    return in_maps


def kernel(text_features, acoustic_features, Wt, bt, Wa, ba, Wq, bq, Wk, bk,
           Wv, bv, **_unused):
    nc = _get_program()
    in_maps = make_in_maps(text_features, acoustic_features, Wt, bt, Wa, ba,
                           Wv, bv)
    res = run_bass_kernel_spmd(nc, in_maps, list(range(N_CORES))).results
    out = np.empty((B, S, D), dtype=np.float32)
    for b in range(N_CORES):
        out[b] = res[b]["out"]
    return out

